# revision 25
# baseline (speedup 1.0000x reference)
"""Causal multi-head attention on 8 Trainium2 NeuronCores.

Problem: B=2, S=2048, H=1024, NH=16, HD=64, fp32 in/out.
Sharding: tensor-parallel over heads (2 heads/core) + AllToAll so every core
computes the output projection for its own 512-token slice.

All layout transforms run on the HOST (numpy): x, Wq/Wk/Wv, Wo arrive
pre-transposed, pre-tiled and bf16, so the PE does no input transposes.
Bias algebra (host):
  - bk drops out: softmax_k[(q+bq)@(k+bk)] == softmax_k[(q+bq)@k]
  - bv folds into the output bias: bo' = Wo @ bv + bo (attn weights sum to 1)
  - bq is applied on the q PSUM->SBUF evacuation (DVE tensor_scalar_add)
  - bo' is applied on the out-proj evacuation (out is computed transposed,
    [H_out, tok], so bo' is a per-partition scalar)

Attention per (chunk, head): S^T[k,q] tiles on PE (causal-narrowed), exp on
ACT -> bf16 P, upper-tri mask on the diagonal tile via DVE mul, then ctx in
the [q, chan] orientation: ctx_t[q, 65] += P_slice.T @ [V_h | 1] per
(kt, q-subtile). That uses all 128 PSUM partitions (half the PE rows of the
[chan, q] form) and makes the softmax denominator a per-partition scalar:
normalize = DVE reciprocal[128,4] + tensor_scalar_mul, no partition
broadcast. A PE transpose flips the normalized [q,64] tiles to [64,512] for
the AllToAll payload.

Schedule per core c (heads 2c, 2c+1):
  P1. per chunk: QKV projection (q/k [chan,tok], V natural into v1 blocks)
      + head-0 attention. Head-1 S+exp for chunks {2,3} run during chunks
      {4,5} (stored P in SBUF) to balance ACT vs PE.
  X0. AllToAll of head-0 ctx; head-1 S+exp for {6,7} + all remaining head-1
      work (P2) overlap it.
  X1. AllToAll of head-1 ctx. Dummy PE matmuls keep the tensor engine at
      peak p-state through the collective windows.
  E.  outT[o,t] = Wo^T.T @ ctx_all + bo' per o-tile; DMA out; host
      transposes/concatenates. Collectives and their 15us fixed cost
      dominate the tail, so there are exactly two.

Tile emission uses a deferred queue: each chunk's final ctx partials and its
normalize chain are emitted after the NEXT chunk's leading matmuls, so the
in-order engine queues never head-block on the exp->ctx->normalize chain.
"""
import sys

if '/opt/trn_rl_repo' not in sys.path:
    sys.path.insert(0, '/opt/trn_rl_repo')

import numpy as np
import ml_dtypes

import concourse.bacc as bacc
import concourse.bass as bass
import concourse.mybir as mybir
from concourse.tile import TileContext
from concourse.bass_utils import run_bass_kernel_spmd
from concourse.masks import make_upper_triangular, make_identity

F32 = mybir.dt.float32
BF16 = mybir.dt.bfloat16
EXP = mybir.ActivationFunctionType.Exp
BF = ml_dtypes.bfloat16

B, S, H, NH, HD = 2, 2048, 1024, 16, 64
NC = 8
T = B * S                 # 4096 tokens
TC = 512                  # tokens per chunk
NCHUNK = T // TC          # 8
NTT = T // 128            # 32 token (k-)tiles
HT = H // 128             # 8 H-tiles
SCALE = 1.0 / np.sqrt(HD)
N_DUMMY = 240             # PE keep-warm matmuls through the AllToAll window

_cache = {}

AHEAD = 2


def _chunk_kts(ch):
    """Per-kt (global k-tile, col offset, width, s) for chunk ch."""
    b, lc = ch // 4, ch % 4
    out = []
    for kt in range(4 * lc + 4):
        s = kt - 4 * lc
        c0 = 128 * s if s >= 0 else 0
        out.append((16 * b + kt, c0, 512 - c0, s))
    return out


class Pipe:
    """Deferred-emission queue (closures emitted later, in order)."""
    def __init__(self):
        self.pending = []

    def defer(self, fn):
        self.pending.append(fn)

    def flush(self):
        while self.pending:
            self.pending.pop(0)()


def _build(phases='LE'):
    key = ('nc', phases)
    if key in _cache:
        return _cache[key]
    nc = bacc.Bacc('TRN2', target_bir_lowering=False, debug=False, num_devices=NC)

    xt_d = nc.dram_tensor('xt', [128, HT * T], BF16, kind='ExternalInput')
    wq_d = nc.dram_tensor('wq', [128, H], BF16, kind='ExternalInput')
    wk_d = nc.dram_tensor('wk', [128, H], BF16, kind='ExternalInput')
    wv_d = nc.dram_tensor('wv', [128, H], BF16, kind='ExternalInput')
    wo_d = nc.dram_tensor('wo', [128, H * HT], BF16, kind='ExternalInput')
    bq_d = nc.dram_tensor('bq', [128, 1], F32, kind='ExternalInput')
    boe_d = nc.dram_tensor('boe', [128, HT], F32, kind='ExternalInput')
    out_d = nc.dram_tensor('out', [H, TC], F32, kind='ExternalOutput')

    with TileContext(nc) as tc:
        with tc.tile_pool(name='persist', bufs=1) as pp, \
             tc.tile_pool(name='scr', bufs=1) as sc, \
             tc.tile_pool(name='dram', bufs=1, space='DRAM') as dpool, \
             tc.tile_pool(name='psum', bufs=1, space='PSUM') as qpool:

            def ptile(shape, dt, tag):
                return pp.tile(shape, dt, tag=tag, name=tag)

            # ---- persistent SBUF ----
            wq_sb = ptile([128, H], BF16, 'wq_sb')
            wk_sb = ptile([128, H], BF16, 'wk_sb')
            wv_sb = ptile([128, H], BF16, 'wv_sb')
            bq_sb = ptile([128, 1], F32, 'bq_sb')
            boe_sb = ptile([128, HT], F32, 'boe_sb')
            nc.sync.dma_start(wq_sb[:], wq_d[:])
            nc.sync.dma_start(wk_sb[:], wk_d[:])
            nc.sync.dma_start(wv_sb[:], wv_d[:])
            nc.sync.dma_start(bq_sb[:], bq_d[:])
            nc.sync.dma_start(boe_sb[:], boe_d[:])

            ut_f = ptile([128, 128], F32, 'ut_f')
            make_upper_triangular(nc, ut_f[:], val=1.0, diag=True)
            ut = ptile([128, 128], BF16, 'ut')
            nc.vector.tensor_copy(ut[:], ut_f[:])
            id_f = ptile([128, 128], F32, 'id_f')
            make_identity(nc, id_f[:])

            qT = ptile([128, T], BF16, 'qT')
            kT = ptile([128, T], BF16, 'kT')
            v1 = ptile([128, NTT * 130], BF16, 'v1')
            woT_sb = ptile([128, H * HT], BF16, 'woT_sb')
            ctxa = ptile([128, NC * TC], BF16, 'ctxa')

            a2a_in = dpool.tile([NCHUNK, 128, TC], BF16)
            a2a_out = dpool.tile([NCHUNK, 128, TC], BF16)

            # v1 ones columns (col 64 + 129 of each 130-block)
            ones_dst = bass.AP(v1.tensor, v1.offset + 64,
                               [list(v1.ap[0]), [130, NTT], [65, 2]])
            nc.vector.memset(ones_dst, 1.0)

            def load_x(ch):
                xs = sc.tile([128, HT * TC], BF16, tag='xs', bufs=3, name='xs')
                src = xt_d[:, :]
                nc.sync.dma_start(
                    xs[:].rearrange('p (i t) -> p i t', i=HT),
                    bass.AP(src.tensor, src.offset + TC * ch,
                            [list(src.ap[0]), [T, HT], [1, TC]]))
                return xs

            def qkv(ch, xs):
                for w_sb, dst, bias in ((wq_sb, qT, bq_sb), (wk_sb, kT, None)):
                    ps = qpool.tile([128, 512], F32, tag='work', bufs=2, name='work')
                    for i in range(HT):
                        nc.tensor.matmul(
                            ps[:], w_sb[:, 128 * i:128 * (i + 1)],
                            xs[:, TC * i:TC * (i + 1)],
                            start=(i == 0), stop=(i == HT - 1))
                    if bias is not None:
                        nc.vector.tensor_scalar_add(
                            dst[:, TC * ch:TC * (ch + 1)], ps[:], bias[:, 0:1])
                    else:
                        nc.vector.tensor_copy(dst[:, TC * ch:TC * (ch + 1)], ps[:])
                # V natural [tok, chan], 4 token tiles side by side in PSUM
                vp = qpool.tile([128, 512], F32, tag='work', bufs=2, name='work')
                for tt in range(4):
                    for i in range(HT):
                        nc.tensor.matmul(
                            vp[:, 128 * tt:128 * (tt + 1)],
                            xs[:, TC * i + 128 * tt:TC * i + 128 * (tt + 1)],
                            wv_sb[:, 128 * i:128 * (i + 1)],
                            start=(i == 0), stop=(i == HT - 1))
                for tt in range(4):
                    kt = 4 * ch + tt
                    base = 130 * kt
                    dst = bass.AP(v1.tensor, v1.offset + base,
                                  [list(v1.ap[0]), [65, 2], [1, 64]])
                    nc.vector.tensor_copy(
                        dst,
                        vp[:, 128 * tt:128 * (tt + 1)].rearrange(
                            'p (g c) -> p g c', g=2))

            def emit_s_pair(ch, h, kts, pi):
                """One [128,1024] PSUM tile holding S for kts 2i, 2i+1."""
                st = qpool.tile([128, 1024], F32, tag='st', bufs=2, name='st')
                for half in range(2):
                    g, c0, w, _ = kts[2 * pi + half]
                    b = 512 * half
                    nc.tensor.matmul(
                        st[:, b + c0:b + 512],
                        kT[64 * h:64 * (h + 1), 128 * g:128 * (g + 1)],
                        qT[64 * h:64 * (h + 1),
                           TC * ch + c0:TC * (ch + 1)],
                        start=True, stop=True)
                return st

            def _ctx_all(ctxt, ch, h, kts, pslice):
                # PSUM accumulation groups must be CONSECUTIVE per bank:
                # finish each qt region before starting the next.
                lc = ch % 4
                for qt in range(4):
                    for kt in range(4 * lc + qt + 1):
                        g, c0, _, _ = kts[kt]
                        nc.tensor.matmul(
                            ctxt[:, 65 * qt:65 * (qt + 1)],
                            pslice(kt, c0, qt),
                            v1[:, 130 * g + 65 * h:130 * g + 65 * h + 65],
                            start=(kt == 0), stop=(kt == 4 * lc + qt))

            def attn(pipe, ch, h):
                """Emit S+exp for (ch, h); ctx accumulation and the
                normalize chain are pushed onto pipe. Previously deferred
                work is flushed once this call's leading S matmuls are
                emitted, so in-order engine queues never head-block on a
                chunk's trailing chain."""
                kts = _chunk_kts(ch)
                npair = len(kts) // 2
                ctxt = qpool.tile([128, 260], F32, tag='ctxt', bufs=2,
                                  name='ctxt')
                sts = {0: emit_s_pair(ch, h, kts, 0)}
                if npair > 1:
                    sts[1] = emit_s_pair(ch, h, kts, 1)
                pipe.flush()
                ptiles = {}
                for pi in range(npair):
                    st = sts.pop(pi)
                    p = sc.tile([128, 1024], BF16, tag='p', bufs=9, name='p')
                    ptiles[pi] = p
                    c0a = kts[2 * pi][1]
                    c0b = kts[2 * pi + 1][1]
                    if c0b <= 128:
                        # one activation spanning both banks (any garbage
                        # cols land in never-read P columns)
                        nc.scalar.activation(p[:, c0a:1024], st[:, c0a:1024],
                                             EXP, scale=float(SCALE))
                    else:
                        nc.scalar.activation(p[:, c0a:512], st[:, c0a:512],
                                             EXP, scale=float(SCALE))
                        nc.scalar.activation(p[:, 512 + c0b:1024],
                                             st[:, 512 + c0b:1024],
                                             EXP, scale=float(SCALE))
                    for half in range(2):
                        s = kts[2 * pi + half][3]
                        if s >= 0:
                            c0 = kts[2 * pi + half][1]
                            pm = p[:, 512 * half + c0:512 * half + c0 + 128]
                            nc.vector.tensor_mul(pm, pm, ut[:])
                    if pi + 2 < npair:
                        sts[pi + 2] = emit_s_pair(ch, h, kts, pi + 2)
                pipe.defer(lambda: _ctx_all(
                    ctxt, ch, h, kts,
                    lambda kt, c0, qt: ptiles[kt // 2][
                        :, 512 * (kt % 2) + 128 * qt:
                        512 * (kt % 2) + 128 * qt + 128]))
                pipe.defer(lambda: _normalize(ctxt, ch, h))

            def _normalize(ctxt, ch, h):
                recip4 = sc.tile([128, 4], F32, tag='recip4', bufs=2,
                                 name='recip4')
                den = bass.AP(ctxt.tensor, ctxt.offset + 64,
                              [list(ctxt.ap[0]), [65, 4]])
                nc.vector.reciprocal(recip4[:], den)
                ctxn = sc.tile([128, 256], F32, tag='ctxn', bufs=2,
                               name='ctxn')
                for qt in range(4):
                    nc.vector.tensor_scalar_mul(
                        ctxn[:, 64 * qt:64 * (qt + 1)],
                        ctxt[:, 65 * qt:65 * qt + 64], recip4[:, qt:qt + 1])
                xp = qpool.tile([128, 1024], F32, tag='st', bufs=2, name='st')
                for qt in range(4):
                    nc.tensor.transpose(xp[0:64, 128 * qt:128 * (qt + 1)],
                                        ctxn[:, 64 * qt:64 * (qt + 1)], id_f[:])
                ctx_sb = sc.tile([64, 512], BF16, tag='ctx_sb', bufs=3,
                                 name='ctx_sb')
                nc.vector.tensor_copy(ctx_sb[:], xp[0:64, 0:512])
                nc.sync.dma_start(a2a_in[ch, 64 * h:64 * (h + 1), :], ctx_sb[:])

            # ---- compute: per chunk QKV + both heads' attention ----
            if 'L' in phases:
                pipe = Pipe()
                next_xs = load_x(0)
                for ch in range(NCHUNK):
                    xs = next_xs
                    if ch + 1 < NCHUNK:
                        next_xs = load_x(ch + 1)
                    if ch == 1:
                        nc.sync.dma_start(woT_sb[:], wo_d[:])
                    qkv(ch, xs)
                    attn(pipe, ch, 0)
                    attn(pipe, ch, 1)
                pipe.flush()

                # ---- X: single AllToAll for both heads ----
                nc.gpsimd.collective_compute(
                    'AllToAll', mybir.AluOpType.bypass,
                    replica_groups=[list(range(NC))],
                    ins=[a2a_in[:]], outs=[a2a_out[:]],
                )
                for j in range(NC):
                    nc.sync.dma_start(ctxa[:, TC * j:TC * (j + 1)],
                                      a2a_out[j, :, :])

                # keep PE at peak p-state through the AllToAll window
                for _ in range(N_DUMMY):
                    dm = qpool.tile([128, 512], F32, tag='work', bufs=2,
                                    name='work')
                    nc.tensor.matmul(dm[:], wq_sb[:, 0:128], wq_sb[:, 0:512],
                                     start=True, stop=True)

            # ---- E: transposed output projection for my 512 tokens ----
            if 'E' in phases:
                for ot in range(HT):
                    ps = qpool.tile([128, 512], F32, tag='work', bufs=2, name='work')
                    for j in range(NC):
                        nc.tensor.matmul(
                            ps[:],
                            woT_sb[:, H * j + 128 * ot:H * j + 128 * (ot + 1)],
                            ctxa[:, TC * j:TC * (j + 1)],
                            start=(j == 0), stop=(j == NC - 1))
                    o_sb = sc.tile([128, 512], F32, tag='o_sb', bufs=2, name='o_sb')
                    nc.vector.tensor_scalar_add(o_sb[:], ps[:],
                                                boe_sb[:, ot:ot + 1])
                    nc.sync.dma_start(out_d[128 * ot:128 * (ot + 1), :], o_sb[:])

    nc.compile()
    _cache[key] = nc
    return nc


def kernel(hidden_states, Wq, bq, Wk, bk, Wv, bv, Wo, bo, **run_kwargs):
    nc = _build()
    hs = np.asarray(hidden_states, np.float32).reshape(T, H)
    Wq, Wk, Wv, Wo = (np.asarray(w, np.float32) for w in (Wq, Wk, Wv, Wo))
    bq, bk, bv, bo = (np.asarray(b, np.float32) for b in (bq, bk, bv, bo))

    def pack(wT):
        # [H_in, C] -> [128, HT*C]: row p holds H-tiles side by side
        c = wT.shape[1]
        return np.ascontiguousarray(
            wT.reshape(HT, 128, c).transpose(1, 0, 2).reshape(128, HT * c)
        ).astype(BF)

    xt = pack(hs.T.copy())
    woT = pack(Wo.T.copy())
    bo_eff = (Wo @ bv + bo).astype(np.float32)
    boe = np.ascontiguousarray(bo_eff.reshape(HT, 128).T)
    in_maps = []
    for c in range(NC):
        r = slice(128 * c, 128 * (c + 1))
        in_maps.append({
            'xt': xt,
            'wq': pack(Wq[r].T.copy()),
            'wk': pack(Wk[r].T.copy()),
            'wv': pack(Wv[r].T.copy()),
            'wo': woT,
            'bq': np.ascontiguousarray(bq[r].reshape(128, 1)),
            'boe': boe,
        })
    res = run_bass_kernel_spmd(nc, in_maps, core_ids=list(range(NC)), **run_kwargs)
    out = np.concatenate([res.results[c]['out'].T for c in range(NC)], axis=0)
    kernel.last_results = res
    return out.reshape(B, S, H)


# revision 26
# speedup vs baseline: 1.0572x; 1.0572x over previous
"""Causal multi-head attention on 8 Trainium2 NeuronCores.

Problem: B=2, S=2048, H=1024, NH=16, HD=64, fp32 in/out.
Sharding: tensor-parallel over heads (2 heads/core) + AllToAll so every core
computes the output projection for its own 512-token slice.

All layout transforms run on the HOST (numpy): x, Wq/Wk/Wv, Wo arrive
pre-transposed, pre-tiled and bf16, so the PE does no input transposes.
Bias algebra (host):
  - bk drops out: softmax_k[(q+bq)@(k+bk)] == softmax_k[(q+bq)@k]
  - bv folds into the output bias: bo' = Wo @ bv + bo (attn weights sum to 1)
  - bq is applied on the q PSUM->SBUF evacuation (DVE tensor_scalar_add)
  - bo' is applied on the out-proj evacuation (out is computed transposed,
    [H_out, tok], so bo' is a per-partition scalar)

Attention per (chunk, head): S^T[k,q] tiles on PE (causal-narrowed), exp on
ACT -> bf16 P, upper-tri mask on the diagonal tile via DVE mul, then ctx in
the [q, chan] orientation: ctx_t[q, 65] += P_slice.T @ [V_h | 1] per
(kt, q-subtile). That uses all 128 PSUM partitions (half the PE rows of the
[chan, q] form) and makes the softmax denominator a per-partition scalar:
normalize = DVE reciprocal[128,4] + tensor_scalar_mul, no partition
broadcast. A PE transpose flips the normalized [q,64] tiles to [64,512] for
the AllToAll payload.

Schedule per core c (heads 2c, 2c+1):
  P1. per chunk: QKV projection (q/k [chan,tok], V natural into v1 blocks)
      + head-0 attention. Head-1 S+exp for chunks {2,3} run during chunks
      {4,5} (stored P in SBUF) to balance ACT vs PE.
  X0. AllToAll of head-0 ctx; head-1 S+exp for {6,7} + all remaining head-1
      work (P2) overlap it.
  X1. AllToAll of head-1 ctx. Dummy PE matmuls keep the tensor engine at
      peak p-state through the collective windows.
  E.  outT[o,t] = Wo^T.T @ ctx_all + bo' per o-tile; DMA out; host
      transposes/concatenates. Collectives and their 15us fixed cost
      dominate the tail, so there are exactly two.

Tile emission uses a deferred queue: each chunk's final ctx partials and its
normalize chain are emitted after the NEXT chunk's leading matmuls, so the
in-order engine queues never head-block on the exp->ctx->normalize chain.
"""
import sys

if '/opt/trn_rl_repo' not in sys.path:
    sys.path.insert(0, '/opt/trn_rl_repo')

import numpy as np
import ml_dtypes

import concourse.bacc as bacc
import concourse.bass as bass
import concourse.mybir as mybir
from concourse.tile import TileContext
from concourse.bass_utils import run_bass_kernel_spmd
from concourse.masks import make_upper_triangular, make_identity

F32 = mybir.dt.float32
F32R = mybir.dt.float32r
BF16 = mybir.dt.bfloat16
EXP = mybir.ActivationFunctionType.Exp
BF = ml_dtypes.bfloat16

B, S, H, NH, HD = 2, 2048, 1024, 16, 64
NC = 8
T = B * S                 # 4096 tokens
TC = 512                  # tokens per chunk
NCHUNK = T // TC          # 8
NTT = T // 128            # 32 token (k-)tiles
HT = H // 128             # 8 H-tiles
SCALE = 1.0 / np.sqrt(HD)
N_DUMMY = 270             # PE keep-warm matmuls through the AllToAll window

_cache = {}

AHEAD = 2


def _chunk_kts(ch):
    """Per-kt (global k-tile, col offset, width, s) for chunk ch."""
    b, lc = ch // 4, ch % 4
    out = []
    for kt in range(4 * lc + 4):
        s = kt - 4 * lc
        c0 = 128 * s if s >= 0 else 0
        out.append((16 * b + kt, c0, 512 - c0, s))
    return out


class Pipe:
    """Deferred-emission queue (closures emitted later, in order)."""
    def __init__(self):
        self.pending = []

    def defer(self, fn):
        self.pending.append(fn)

    def flush(self):
        while self.pending:
            self.pending.pop(0)()


def _build(phases='LE'):
    key = ('nc', phases)
    if key in _cache:
        return _cache[key]
    nc = bacc.Bacc('TRN2', target_bir_lowering=False, debug=False, num_devices=NC)

    xt_d = nc.dram_tensor('xt', [128, HT * T], BF16, kind='ExternalInput')
    wq_d = nc.dram_tensor('wq', [128, H], BF16, kind='ExternalInput')
    wk_d = nc.dram_tensor('wk', [128, H], BF16, kind='ExternalInput')
    wv_d = nc.dram_tensor('wv', [128, H], BF16, kind='ExternalInput')
    wo_d = nc.dram_tensor('wo', [128, H * HT], BF16, kind='ExternalInput')
    bq_d = nc.dram_tensor('bq', [128, 1], F32, kind='ExternalInput')
    boe_d = nc.dram_tensor('boe', [128, HT], F32, kind='ExternalInput')
    out_d = nc.dram_tensor('out', [H, TC], F32, kind='ExternalOutput')

    with TileContext(nc) as tc:
        with tc.tile_pool(name='persist', bufs=1) as pp, \
             tc.tile_pool(name='scr', bufs=1) as sc, \
             tc.tile_pool(name='dram', bufs=1, space='DRAM') as dpool, \
             tc.tile_pool(name='psum', bufs=1, space='PSUM') as qpool:

            def ptile(shape, dt, tag):
                return pp.tile(shape, dt, tag=tag, name=tag)

            # ---- persistent SBUF ----
            wq_sb = ptile([128, H], BF16, 'wq_sb')
            wk_sb = ptile([128, H], BF16, 'wk_sb')
            wv_sb = ptile([128, H], BF16, 'wv_sb')
            bq_sb = ptile([128, 1], F32, 'bq_sb')
            boe_sb = ptile([128, HT], F32, 'boe_sb')
            nc.sync.dma_start(wq_sb[:], wq_d[:])
            nc.sync.dma_start(wk_sb[:], wk_d[:])
            nc.sync.dma_start(wv_sb[:], wv_d[:])
            nc.sync.dma_start(bq_sb[:], bq_d[:])
            nc.sync.dma_start(boe_sb[:], boe_d[:])

            ut_f = ptile([128, 128], F32, 'ut_f')
            make_upper_triangular(nc, ut_f[:], val=1.0, diag=True)
            ut = ptile([128, 128], BF16, 'ut')
            nc.vector.tensor_copy(ut[:], ut_f[:])
            id_f = ptile([128, 128], F32, 'id_f')
            make_identity(nc, id_f[:])
            id_r = ptile([128, 128], F32R, 'id_r')
            nc.vector.tensor_copy(id_r[:], id_f[:])

            qT = ptile([128, T], BF16, 'qT')
            kT = ptile([128, T], BF16, 'kT')
            v1 = ptile([128, NTT * 130], BF16, 'v1')
            woT_sb = ptile([128, H * HT], BF16, 'woT_sb')
            ctxa = ptile([128, NC * TC], BF16, 'ctxa')

            a2a_in = dpool.tile([NCHUNK, 128, TC], BF16)
            a2a_out = dpool.tile([NCHUNK, 128, TC], BF16)

            # v1 ones columns (col 64 + 129 of each 130-block)
            ones_dst = bass.AP(v1.tensor, v1.offset + 64,
                               [list(v1.ap[0]), [130, NTT], [65, 2]])
            nc.vector.memset(ones_dst, 1.0)

            def load_x(ch):
                xs = sc.tile([128, HT * TC], BF16, tag='xs', bufs=3, name='xs')
                src = xt_d[:, :]
                nc.sync.dma_start(
                    xs[:].rearrange('p (i t) -> p i t', i=HT),
                    bass.AP(src.tensor, src.offset + TC * ch,
                            [list(src.ap[0]), [T, HT], [1, TC]]))
                return xs

            def qkv(ch, xs):
                for w_sb, dst, bias in ((wq_sb, qT, bq_sb), (wk_sb, kT, None)):
                    ps = qpool.tile([128, 512], F32, tag='work', bufs=2, name='work')
                    for i in range(HT):
                        nc.tensor.matmul(
                            ps[:], w_sb[:, 128 * i:128 * (i + 1)],
                            xs[:, TC * i:TC * (i + 1)],
                            start=(i == 0), stop=(i == HT - 1))
                    if bias is not None:
                        nc.vector.tensor_scalar_add(
                            dst[:, TC * ch:TC * (ch + 1)], ps[:], bias[:, 0:1])
                    else:
                        nc.vector.tensor_copy(dst[:, TC * ch:TC * (ch + 1)], ps[:])
                # V natural [tok, chan], 4 token tiles side by side in PSUM
                vp = qpool.tile([128, 512], F32, tag='work', bufs=2, name='work')
                for tt in range(4):
                    for i in range(HT):
                        nc.tensor.matmul(
                            vp[:, 128 * tt:128 * (tt + 1)],
                            xs[:, TC * i + 128 * tt:TC * i + 128 * (tt + 1)],
                            wv_sb[:, 128 * i:128 * (i + 1)],
                            start=(i == 0), stop=(i == HT - 1))
                for tt in range(4):
                    kt = 4 * ch + tt
                    base = 130 * kt
                    dst = bass.AP(v1.tensor, v1.offset + base,
                                  [list(v1.ap[0]), [65, 2], [1, 64]])
                    nc.vector.tensor_copy(
                        dst,
                        vp[:, 128 * tt:128 * (tt + 1)].rearrange(
                            'p (g c) -> p g c', g=2))

            def emit_s_pair(ch, h, kts, pi):
                """One [128,1024] PSUM tile holding S for kts 2i, 2i+1."""
                st = qpool.tile([128, 1024], F32, tag='st', bufs=2, name='st')
                for half in range(2):
                    g, c0, w, _ = kts[2 * pi + half]
                    b = 512 * half
                    nc.tensor.matmul(
                        st[:, b + c0:b + 512],
                        kT[64 * h:64 * (h + 1), 128 * g:128 * (g + 1)],
                        qT[64 * h:64 * (h + 1),
                           TC * ch + c0:TC * (ch + 1)],
                        start=True, stop=True)
                return st

            def _ctx_all(ctxt, ch, h, kts, pslice):
                # PSUM accumulation groups must be CONSECUTIVE per bank:
                # finish each qt region before starting the next.
                lc = ch % 4
                for qt in range(4):
                    for kt in range(4 * lc + qt + 1):
                        g, c0, _, _ = kts[kt]
                        nc.tensor.matmul(
                            ctxt[:, 65 * qt:65 * (qt + 1)],
                            pslice(kt, c0, qt),
                            v1[:, 130 * g + 65 * h:130 * g + 65 * h + 65],
                            start=(kt == 0), stop=(kt == 4 * lc + qt))

            def attn(pipe, ch, h):
                """Emit S+exp for (ch, h); ctx accumulation and the
                normalize chain are pushed onto pipe. Previously deferred
                work is flushed once this call's leading S matmuls are
                emitted, so in-order engine queues never head-block on a
                chunk's trailing chain."""
                kts = _chunk_kts(ch)
                npair = len(kts) // 2
                ctxt = qpool.tile([128, 260], F32, tag='ctxt', bufs=1,
                                  name='ctxt')
                sts = {0: emit_s_pair(ch, h, kts, 0)}
                if npair > 1:
                    sts[1] = emit_s_pair(ch, h, kts, 1)
                pipe.flush()
                ptiles = {}
                for pi in range(npair):
                    st = sts.pop(pi)
                    p = sc.tile([128, 1024], BF16, tag='p', bufs=9, name='p')
                    ptiles[pi] = p
                    c0a = kts[2 * pi][1]
                    c0b = kts[2 * pi + 1][1]
                    if c0b <= 128:
                        # one activation spanning both banks (any garbage
                        # cols land in never-read P columns)
                        nc.scalar.activation(p[:, c0a:1024], st[:, c0a:1024],
                                             EXP, scale=float(SCALE))
                    else:
                        nc.scalar.activation(p[:, c0a:512], st[:, c0a:512],
                                             EXP, scale=float(SCALE))
                        nc.scalar.activation(p[:, 512 + c0b:1024],
                                             st[:, 512 + c0b:1024],
                                             EXP, scale=float(SCALE))
                    for half in range(2):
                        s = kts[2 * pi + half][3]
                        if s >= 0:
                            c0 = kts[2 * pi + half][1]
                            pm = p[:, 512 * half + c0:512 * half + c0 + 128]
                            nc.vector.tensor_mul(pm, pm, ut[:])
                    if pi + 2 < npair:
                        sts[pi + 2] = emit_s_pair(ch, h, kts, pi + 2)
                pipe.defer(lambda: _ctx_all(
                    ctxt, ch, h, kts,
                    lambda kt, c0, qt: ptiles[kt // 2][
                        :, 512 * (kt % 2) + 128 * qt:
                        512 * (kt % 2) + 128 * qt + 128]))
                pipe.defer(lambda: _normalize(ctxt, ch, h))

            def _normalize(ctxt, ch, h):
                recip4 = sc.tile([128, 4], F32, tag='recip4', bufs=2,
                                 name='recip4')
                den = bass.AP(ctxt.tensor, ctxt.offset + 64,
                              [list(ctxt.ap[0]), [65, 4]])
                nc.vector.reciprocal(recip4[:], den)
                ctxn = sc.tile([128, 256], F32R, tag='ctxn', bufs=2,
                               name='ctxn')
                for qt in range(4):
                    nc.vector.tensor_scalar_mul(
                        ctxn[:, 64 * qt:64 * (qt + 1)],
                        ctxt[:, 65 * qt:65 * qt + 64], recip4[:, qt:qt + 1])
                xp = qpool.tile([128, 512], F32R, tag='xp', bufs=1, name='xp')
                for qt in range(4):
                    nc.tensor.transpose(xp[0:64, 128 * qt:128 * (qt + 1)],
                                        ctxn[:, 64 * qt:64 * (qt + 1)], id_r[:])
                ctx_sb = sc.tile([64, 512], BF16, tag='ctx_sb', bufs=3,
                                 name='ctx_sb')
                nc.vector.tensor_copy(ctx_sb[:], xp[0:64, 0:512])
                nc.sync.dma_start(a2a_in[ch, 64 * h:64 * (h + 1), :], ctx_sb[:])

            # ---- compute: per chunk QKV + both heads' attention ----
            if 'L' in phases:
                pipe = Pipe()
                next_xs = load_x(0)
                for ch in range(NCHUNK):
                    xs = next_xs
                    if ch + 1 < NCHUNK:
                        next_xs = load_x(ch + 1)
                    if ch == 1:
                        nc.sync.dma_start(woT_sb[:], wo_d[:])
                    qkv(ch, xs)
                    attn(pipe, ch, 0)
                    attn(pipe, ch, 1)
                pipe.flush()

                # ---- X: single AllToAll for both heads ----
                nc.gpsimd.collective_compute(
                    'AllToAll', mybir.AluOpType.bypass,
                    replica_groups=[list(range(NC))],
                    ins=[a2a_in[:]], outs=[a2a_out[:]],
                )
                for j in range(NC):
                    nc.sync.dma_start(ctxa[:, TC * j:TC * (j + 1)],
                                      a2a_out[j, :, :])

                # keep PE at peak p-state through the AllToAll window
                for _ in range(N_DUMMY):
                    dm = qpool.tile([128, 512], F32, tag='work', bufs=2,
                                    name='work')
                    nc.tensor.matmul(dm[:], wq_sb[:, 0:128], wq_sb[:, 0:512],
                                     start=True, stop=True)

            # ---- E: transposed output projection for my 512 tokens ----
            if 'E' in phases:
                for ot in range(HT):
                    ps = qpool.tile([128, 512], F32, tag='work', bufs=2, name='work')
                    for j in range(NC):
                        nc.tensor.matmul(
                            ps[:],
                            woT_sb[:, H * j + 128 * ot:H * j + 128 * (ot + 1)],
                            ctxa[:, TC * j:TC * (j + 1)],
                            start=(j == 0), stop=(j == NC - 1))
                    o_sb = sc.tile([128, 512], F32, tag='o_sb', bufs=2, name='o_sb')
                    nc.vector.tensor_scalar_add(o_sb[:], ps[:],
                                                boe_sb[:, ot:ot + 1])
                    nc.sync.dma_start(out_d[128 * ot:128 * (ot + 1), :], o_sb[:])

    nc.compile()
    _cache[key] = nc
    return nc


def kernel(hidden_states, Wq, bq, Wk, bk, Wv, bv, Wo, bo, **run_kwargs):
    nc = _build()
    hs = np.asarray(hidden_states, np.float32).reshape(T, H)
    Wq, Wk, Wv, Wo = (np.asarray(w, np.float32) for w in (Wq, Wk, Wv, Wo))
    bq, bk, bv, bo = (np.asarray(b, np.float32) for b in (bq, bk, bv, bo))

    def pack(wT):
        # [H_in, C] -> [128, HT*C]: row p holds H-tiles side by side
        c = wT.shape[1]
        return np.ascontiguousarray(
            wT.reshape(HT, 128, c).transpose(1, 0, 2).reshape(128, HT * c)
        ).astype(BF)

    xt = pack(hs.T.copy())
    woT = pack(Wo.T.copy())
    bo_eff = (Wo @ bv + bo).astype(np.float32)
    boe = np.ascontiguousarray(bo_eff.reshape(HT, 128).T)
    in_maps = []
    for c in range(NC):
        r = slice(128 * c, 128 * (c + 1))
        in_maps.append({
            'xt': xt,
            'wq': pack(Wq[r].T.copy()),
            'wk': pack(Wk[r].T.copy()),
            'wv': pack(Wv[r].T.copy()),
            'wo': woT,
            'bq': np.ascontiguousarray(bq[r].reshape(128, 1)),
            'boe': boe,
        })
    res = run_bass_kernel_spmd(nc, in_maps, core_ids=list(range(NC)), **run_kwargs)
    out = np.concatenate([res.results[c]['out'].T for c in range(NC)], axis=0)
    kernel.last_results = res
    return out.reshape(B, S, H)
